# revision 40
# baseline (speedup 1.0000x reference)
"""Trainium2 Bass kernel for batched single-head attention with projections.

Reference computation (per batch b):
    Q = q @ Wq + bq ; K = k @ Wk + bk ; V = v @ Wv + bv        (512 -> 64)
    out = softmax(Q K^T / 8) V                                  (S = 4096)

Sharding: 8 cores = 4 batches x 2 kv-sequence halves. Each core gets
its full q (transposed, bf16) plus half of k,v for its batch (transposed,
bf16). Cores emit unnormalized numerator + denominator; host sums the
two kv-half partials per batch and normalizes.

Device-side layout (transposed space):
  Q.T [128, 4096] = (Wq|Wq).T @ qT (+bq)   rows 64..127 duplicate 0..63
  K.T [128, 2048] = (Wk|Wk).T @ kT         (bk dropped: softmax-invariant)
  V'  [2048, 65]  = vT.T @ Wv_aug + bias ; col 64 == 1.0 (denominator col)
  per kv-tile (128 kv rows x 512 q): scores.T -> PSUM, exp -> bf16 SBUF,
  V'.T @ P.T accumulated into [65, 512] per q-block.

Perf structure:
  - ScalarE is the floor engine (~8.5M exps/core at 1 elem/lane/cycle);
    it does exps only. scores PSUM = two 3-bank slots; one ACTIVATE per
    3-tile group (N=1536) ping-ponging the slots.
  - kv-tile pairs row-pack the PE (tile_position) so the 64-contraction
    scores MMs run concurrently; AV keeps full 128-contraction (row-
    splitting it doubles serialized LDWEIGHTS - measured regression).
  - PE warm-up matmuls on memset data flip the HAM clock gate to 2.4 GHz
    before the first input DMA lands.
  - DMA arrival order == need order per HWDGE ring FIFO; the early HBM
    window (~250 GB/s/core with small packets) is the head bottleneck.
    All five weight tensors ride ONE descriptor (bf16 "wall") at the
    head of the scalar ring, followed by q block 0 + V; sync carries the
    K quarters + the q tail + the output stores; the slow-starting SWDGE
    ring is kept empty. AVs lag scores by up to 6 exp-groups (tapering
    to 2) so late V quarters never stall the in-order PE queue.
"""

import numpy as np
import ml_dtypes

import concourse.bass as bass
import concourse.tile as tile
from concourse import mybir
from concourse.bass_utils import run_bass_kernel_spmd

BF16 = mybir.dt.bfloat16
F32 = mybir.dt.float32

B, S, D, E = 4, 4096, 512, 64
H = S                 # q rows per core (full sequence)
KS = S // 2           # kv rows per core (half sequence)
E1 = E + 1            # V' width (ones column appended)
NCH = D // 128        # contraction chunks (4)
NKV = KS // 128       # kv tiles per core (16)
QBLK = 512            # q columns per block
NBLK = H // QBLK      # 8
NT = NBLK * NKV       # global tile-unit count (128)
GRP = 3               # tile-units per exp group / psum slot
NGRP = (NT + GRP - 1) // GRP    # 43 (last group has 2 units)
N_CORES = 8

# weight-wall slice offsets (all bf16): wk, wq, bq, wv, bvb
_W_WK = 0
_W_WQ = 512
_W_BQ = 1024
_W_WV = 1536
_W_BVB = 1536 + NCH * E1
_W_END = _W_BVB + E1            # 1861


def _useg(U):
    """tile-unit U (0..127) -> (q-block, kv-tile)."""
    return divmod(U, NKV)


def _build_bass(split_waits: bool = True) -> bass.Bass:
    nc = bass.Bass()
    qT = nc.declare_dram_parameter("qT", [D, H], BF16, isOutput=False)
    kT = nc.declare_dram_parameter("kT", [D, KS], BF16, isOutput=False)
    vT = nc.declare_dram_parameter("vT", [D, KS], BF16, isOutput=False)
    wall = nc.declare_dram_parameter("wall", [128, _W_END], BF16, isOutput=False)
    out = nc.declare_dram_parameter("out", [E1, H], BF16, isOutput=True)

    with tile.TileContext(nc) as tc:
        _body(nc, tc, qT, kT, vT, wall, out)
    _prune_waits(nc)
    if split_waits:
        _split_multi_waits(nc)
    return nc


_NO_SPLIT_OPCODES = {"Drain", "EventSemaphore", "NoOp", "Call", "ISA",
                     "UnconditionalBranch"}


def _prune_waits(nc):
    """Drop provably-redundant sem waits before the wait-split pass.

    (a) A wait on a sem that is incremented ONLY by the waiting
        instruction's own engine is satisfied by in-order execution
        (tile never emits a forward-referencing wait - that would
        deadlock), so it can go.
    (b) A sem-ge-imm wait dominated by an earlier wait on the same
        engine for the same sem with a >= threshold (counters are
        monotone) can go.
    Fewer multi-wait instructions -> fewer injected EventSemaphore
    stalls on the ScalarE/PE queues."""
    incrementers = {}
    for blk in nc.m.functions[0].blocks:
        for inst in blk.instructions:
            si = inst.sync_info
            if si is not None:
                for u in (si.on_update or []):
                    incrementers.setdefault(u.id, set()).add(str(inst.engine))
    seen_max = {}
    dropped = 0
    for blk in nc.m.functions[0].blocks:
        for inst in blk.instructions:
            si = inst.sync_info
            if si is None or not si.on_wait:
                continue
            eng = str(inst.engine)
            keep = []
            for w in si.on_wait:
                nm = (w.ant_name or "")
                if (w.wait_mode != "sem-ge-imm" or w.wait_value is None
                        or nm.split("_")[0] not in
                        ("PE", "Activation", "DVE", "SP", "Pool")):
                    keep.append(w)
                    continue
                incs = incrementers.get(w.id)
                if incs is not None and incs == {eng}:
                    dropped += 1
                    continue
                prev = seen_max.get((eng, w.id))
                if prev is not None and prev >= w.wait_value:
                    dropped += 1
                    continue
                seen_max[(eng, w.id)] = max(prev or 0, w.wait_value)
                keep.append(w)
            if len(keep) != len(si.on_wait):
                inst.sync_info = mybir.SyncInfo(
                    on_wait=keep, on_update=list(si.on_update or []))
    return dropped


def _split_multi_waits(nc):
    """walrus (this toolchain) encodes at most ONE sem wait per TPB
    instruction (single NEURON_ISA_TPB_EVENTS slot) and refuses to compile
    instructions carrying more. Tile emits multi-wait sync_info freely, so
    split: keep the first wait on the instruction, hoist the rest onto
    standalone EventSemaphore waits just before it on the same engine."""
    n = 0
    for blk in nc.m.functions[0].blocks:
        new_insts = []
        for inst in blk.instructions:
            si = inst.sync_info
            if (si is not None and si.on_wait and len(si.on_wait) > 1
                    and inst.concise_opcode not in _NO_SPLIT_OPCODES):
                waits = list(si.on_wait)
                for w in waits[:-1]:
                    n += 1
                    es = mybir.InstEventSemaphore(
                        name=f"WSPLIT-{n}", ins=[], outs=[])
                    es.engine = inst.engine
                    es.sync_info = mybir.SyncInfo(on_wait=[w], on_update=[])
                    new_insts.append(es)
                inst.sync_info = mybir.SyncInfo(
                    on_wait=[waits[-1]], on_update=list(si.on_update))
            new_insts.append(inst)
        blk.instructions = new_insts
    return nc


def _body(nc, tc, qT, kT, vT, wall, out):
    with (
        tc.tile_pool(name="consts", bufs=1) as cst,
        tc.tile_pool(name="raw", bufs=1) as raw,
        tc.tile_pool(name="proj", bufs=1) as proj,
        tc.tile_pool(name="pt", bufs=12) as ptp,
        tc.tile_pool(name="ob", bufs=2) as obp,
        tc.tile_pool(name="sc", bufs=2, space="PSUM") as scp,
        tc.tile_pool(name="acc", bufs=1, space="PSUM") as accp,
        tc.tile_pool(name="pp", bufs=1, space="PSUM") as ppp,
    ):
        def load3d(eng, name, src, c0, c1):
            w = c1 - c0
            t = raw.tile([128, NCH, w], BF16, tag=name, name=name)
            eng.dma_start(
                out=t,
                in_=src[:, c0:c1].rearrange("(c p) w -> p c w", p=128))
            return t

        # gpsimd (SWDGE): warm-up memset; later only output stores
        warm = cst.tile([128, 256], BF16, tag="warm")
        nc.gpsimd.memset(warm, 0.0)

        # scalar ring (HWDGE): weight wall in ONE descriptor, q block 0,
        # the exp-table preload, q block 1, then all of V; afterwards the
        # ScalarE queue runs exps only.
        kqa = load3d(nc.scalar, "kqa", kT, 0, 256)
        wall_sb = cst.tile([128, _W_END], BF16, tag="wall")
        nc.scalar.dma_start(out=wall_sb, in_=wall[:, :])
        kq1 = load3d(nc.scalar, "kq1", kT, 512, 1024)
        scr = cst.tile([1, 8], F32, tag="scr")
        nc.scalar.activation(scr[:, :], warm[0:1, 0:8],
                             mybir.ActivationFunctionType.Exp)
        qt_b1 = load3d(nc.scalar, "qt_b1", qT, 512, 1024)
        vqs = [load3d(nc.scalar, f"vq{i}", vT, i * 512, (i + 1) * 512)
               for i in range(4)]

        # sync ring (HWDGE): K first half, q tail, K second half; the
        # per-block output stores ride this ring late (SWDGE is too slow)
        qt_b0 = load3d(nc.sync, "qt_b0", qT, 0, 512)
        kqb = load3d(nc.sync, "kqb", kT, 256, 512)
        kq2 = load3d(nc.sync, "kq2", kT, 1024, 1536)
        kq3 = load3d(nc.sync, "kq3", kT, 1536, 2048)
        qt_b2 = load3d(nc.sync, "qt_b2", qT, 1024, 1536)
        qt_r34 = load3d(nc.sync, "qt_r34", qT, 1536, 2560)
        qt_r567 = load3d(nc.sync, "qt_r567", qT, 2560, H)

        wk_sb = wall_sb[:, _W_WK:_W_WK + 512]
        wq_sb = wall_sb[:, _W_WQ:_W_WQ + 512]
        bq_sb = wall_sb[:, _W_BQ:_W_BQ + 512]
        wv_sb = wall_sb[:, _W_WV:_W_WV + NCH * E1]
        bvb_sb = wall_sb[:, _W_BVB:_W_BVB + E1]

        # PE warm-up: ~4us of matmuls on memset garbage flips the HAM
        # clock gate (1.2 -> 2.4 GHz) while the input DMAs are in flight.
        wacc = ppp.tile([128, QBLK], F32, tag="pp", name="wacc")
        for i in range(17):
            nc.tensor.matmul(wacc[:, 0:256], warm[:, 0:128], warm[:, :],
                             start=(i == 0), stop=(i == 16))

        def qt_slice(c, blk):
            if blk == 0:
                return qt_b0[:, c, :]
            if blk == 1:
                return qt_b1[:, c, :]
            if blk == 2:
                return qt_b2[:, c, :]
            if blk < 5:
                return qt_r34[:, c, (blk - 3) * 512:(blk - 2) * 512]
            return qt_r567[:, c, (blk - 5) * 512:(blk - 4) * 512]

        def k_moving(c, kt2b):      # 256-col raw-K slice for projection
            if kt2b == 0:
                return kqa[:, c, :]
            if kt2b == 1:
                return kqb[:, c, :]
            src = (kq1, kq1, kq2, kq2, kq3, kq3)[kt2b - 2]
            off = (kt2b % 2) * 256
            return src[:, c, off:off + 256]

        def vt_slice(c, t):         # 128-col V tile
            return vqs[t // 4][:, c, (t % 4) * 128:(t % 4 + 1) * 128]

        # projected tensors; Q.T/K.T rows 64..127 duplicate rows 0..63 (via
        # column-duplicated weights) so scores matmuls can row-pack.
        QT2 = proj.tile([128, H], BF16, tag="QT2")
        KT2 = proj.tile([128, KS], BF16, tag="KT2")
        Vp = proj.tile([128, NKV, E1], BF16, tag="Vp")

        # AV accumulators ping-pong between the acc bank (blocks
        # 0,1,3,5,7) and the pp bank (blocks 2,4,6) so a block's first AV
        # never waits on the previous block's epilogue drain; q_proj picks
        # whichever bank is free in its emission window.
        def q_pool(blk):
            return accp if blk in (0, 1, 3, 5, 7) else ppp

        def q_proj(blk, pool=None):
            pool = ppp if pool is None else pool
            tg = "acc" if pool is accp else "pp"
            acc = pool.tile([128, QBLK], F32, tag=tg, name="qp")
            sl = slice(blk * QBLK, (blk + 1) * QBLK)
            for c in range(NCH):
                nc.tensor.matmul(
                    acc[:, :], wq_sb[:, c * 128:(c + 1) * 128],
                    qt_slice(c, blk),
                    start=(c == 0), stop=(c == NCH - 1))
            nc.vector.tensor_add(QT2[:, sl], acc[:, :], bq_sb[:, :])

        def k_proj(kt2b):           # 256-col K projection sub-block
            acc = ppp.tile([128, QBLK], F32, tag="pp", name="kp")
            sl = slice(kt2b * 256, (kt2b + 1) * 256)
            for c in range(NCH):
                nc.tensor.matmul(
                    acc[:, 0:256], wk_sb[:, c * 128:(c + 1) * 128],
                    k_moving(c, kt2b),
                    start=(c == 0), stop=(c == NCH - 1))
            nc.vector.tensor_copy(KT2[:, sl], acc[:, 0:256])

        def v_proj(t):
            acc = ppp.tile([128, QBLK], F32, tag="pp", name="vp")
            for c in range(NCH):
                nc.tensor.matmul(
                    acc[:, 0:E1], vt_slice(c, t),
                    wv_sb[:, c * E1:(c + 1) * E1],
                    start=(c == 0), stop=(c == NCH - 1))
            nc.vector.tensor_add(Vp[:, t, :], acc[:, 0:E1], bvb_sb[:, :])

        # --- attention over tile-units U (see _useg). Groups of GRP=3
        # units share one psum slot + one exp ACTIVATE.
        state = {"sc": None, "acc": None}
        pt_of = {}

        def scores(U):
            blk, t = _useg(U)
            p = U % GRP
            if p == 0:
                state["sc"] = scp.tile([128, GRP * QBLK], F32, tag="sc",
                                       name="sc")
            sq = slice(blk * QBLK, (blk + 1) * QBLK)
            half = U % 2
            nc.tensor.matmul(
                state["sc"][:, p * QBLK:(p + 1) * QBLK],
                KT2[half * E:(half + 1) * E, t * 128:(t + 1) * 128],
                QT2[half * E:(half + 1) * E, sq],
                start=True, stop=True, tile_position=(half * E, 0))

        def exp_group(g):
            lastU = min((g + 1) * GRP, NT) - 1
            n = (lastU % GRP) + 1
            pt = ptp.tile([128, GRP * QBLK], BF16, tag="pt", name="pt")
            nc.scalar.activation(
                pt[:, 0:n * QBLK], state["sc"][:, 0:n * QBLK],
                mybir.ActivationFunctionType.Exp, scale=0.125)
            pt_of[g] = pt

        def epilogue(blk):
            sq = slice(blk * QBLK, (blk + 1) * QBLK)
            ob = obp.tile([E1, QBLK], BF16, tag="ob", name="ob")
            nc.vector.tensor_copy(ob[:, :], state["acc"][:, :])
            nc.sync.dma_start(out=out[:, sq], in_=ob[:, :])

        def av(U):
            blk, t = _useg(U)
            if t == 0:
                if blk > 0:
                    epilogue(blk - 1)
                pool = accp if blk in (0, 1, 3, 5, 7) else ppp
                tg = "acc" if pool is accp else "pp"
                state["acc"] = pool.tile([E1, QBLK], F32, tag=tg,
                                         name="acc")
            nc.tensor.matmul(
                state["acc"][:, :], Vp[:, t, :],
                pt_of[U // GRP][:, (U % GRP) * QBLK:(U % GRP + 1) * QBLK],
                start=(t == 0), stop=(t == NKV - 1))

        def sc_group(g):
            for U in range(g * GRP, min((g + 1) * GRP, NT)):
                scores(U)
            exp_group(g)

        def av_group(g):
            for U in range(g * GRP, min((g + 1) * GRP, NT)):
                av(U)

        # --- schedule. K/Q-critical chain first; AVs lag scores by up
        # to 6 groups while V arrives, tapering to 2 near the end. The
        # AV accumulator alternates acc/pp banks per block; q_proj slots
        # into whichever bank is free in its emission window.
        k_proj(0)
        q_proj(0, pool=accp)
        scores(0); scores(1)
        k_proj(1)
        scores(2); exp_group(0)
        k_proj(2); k_proj(3)
        sc_group(1)
        k_proj(4); k_proj(5)
        sc_group(2)
        k_proj(6); k_proj(7)
        sc_group(3)
        sc_group(4)
        q_proj(1, pool=accp)
        sc_group(5)
        sc_group(6)
        v_proj(0); v_proj(1); v_proj(2)

        VP_AT = {7: (3, 4, 5), 8: (6, 7, 8), 9: (9, 10, 11),
                 10: (12, 13, 14, 15)}
        QP_END = {9: 2, 14: 3, 20: 4, 25: 5, 30: 6, 35: 7}
        next_av = 0
        for g in range(7, NGRP):
            for t in VP_AT.get(g, ()):
                v_proj(t)
            sc_group(g)
            lag = 6 if g < 12 else 5 if g < 13 else 4 if g < 26 else \
                3 if g < 29 else 2
            while next_av <= g - lag:
                av_group(next_av)
                next_av += 1
            if g in QP_END:
                q_proj(QP_END[g], pool=q_pool(QP_END[g]))
        while next_av < NGRP:
            av_group(next_av)
            next_av += 1
        epilogue(NBLK - 1)


_CACHED_NC = None


def _get_nc():
    global _CACHED_NC
    if _CACHED_NC is None:
        _CACHED_NC = _build_bass()
    return _CACHED_NC


def _swizzle_w(w: np.ndarray) -> np.ndarray:
    """[512, width] -> [128, NCH*width] with chunk-major free dim."""
    width = w.shape[1]
    return np.ascontiguousarray(
        w.reshape(NCH, 128, width).transpose(1, 0, 2).reshape(128, NCH * width)
    ).astype(ml_dtypes.bfloat16)


def _make_in_maps(q, k, v, Wq, bq, Wk, bk, Wv, bv):
    del bk  # constant along the kv axis -> softmax-invariant, dropped
    bf = ml_dtypes.bfloat16
    wq_d = np.concatenate([np.asarray(Wq, np.float32)] * 2, axis=1)
    wk_d = np.concatenate([np.asarray(Wk, np.float32)] * 2, axis=1)
    wq_s = _swizzle_w(wq_d)
    wk_s = _swizzle_w(wk_d)
    wv_aug = np.concatenate(
        [np.asarray(Wv, np.float32), np.zeros((D, 1), np.float32)], axis=1
    )
    wv_s = _swizzle_w(wv_aug)
    bq_col = np.asarray(bq, np.float32).reshape(E, 1)
    bq_a = np.ascontiguousarray(np.broadcast_to(
        np.concatenate([bq_col, bq_col], axis=0), (2 * E, QBLK))).astype(bf)
    bvb_row = np.concatenate([np.asarray(bv, np.float32), [1.0]])
    bvb_a = np.ascontiguousarray(
        np.broadcast_to(bvb_row, (128, E1))).astype(bf)
    wall = np.concatenate([wk_s, wq_s, bq_a, wv_s, bvb_a], axis=1)
    assert wall.shape == (128, _W_END)

    in_maps = []
    for core in range(N_CORES):
        b, h = core // 2, core % 2
        kh = np.asarray(k[b, h * KS:(h + 1) * KS, :], np.float32)
        vh = np.asarray(v[b, h * KS:(h + 1) * KS, :], np.float32)
        in_maps.append({
            "qT": np.ascontiguousarray(np.asarray(q[b], np.float32).T).astype(bf),
            "kT": np.ascontiguousarray(kh.T).astype(bf),
            "vT": np.ascontiguousarray(vh.T).astype(bf),
            "wall": wall,
        })
    return in_maps


def _unshard(results) -> np.ndarray:
    final = np.empty((B, S, E), np.float32)
    for b in range(B):
        o = (np.asarray(results[2 * b]["out"], np.float32)
             + np.asarray(results[2 * b + 1]["out"], np.float32))  # [65, S]
        final[b] = (o[:E] / o[E:E + 1]).T
    return final


def kernel(q, k, v, Wq, bq, Wk, bk, Wv, bv, _trace=False):
    nc = _get_nc()
    in_maps = _make_in_maps(q, k, v, Wq, bq, Wk, bk, Wv, bv)
    res = run_bass_kernel_spmd(nc, in_maps, core_ids=list(range(N_CORES)),
                               trace=_trace)
    outp = _unshard(res.results)
    if _trace:
        kernel.last_result = res
    return outp


# revision 41
# speedup vs baseline: 1.1764x; 1.1764x over previous
"""Trainium2 Bass kernel for batched single-head attention with projections.

Reference computation (per batch b):
    Q = q @ Wq + bq ; K = k @ Wk + bk ; V = v @ Wv + bv        (512 -> 64)
    out = softmax(Q K^T / 8) V                                  (S = 4096)

Sharding: 8 cores = 4 batches x 2 kv-sequence halves. Each core gets
its full q (transposed, bf16) plus half of k,v for its batch (transposed,
bf16). Cores emit unnormalized numerator + denominator; host sums the
two kv-half partials per batch and normalizes.

Device-side layout (transposed space):
  Q.T [128, 4096] = (Wq|Wq).T @ qT (+bq)   rows 64..127 duplicate 0..63
  K.T [128, 2048] = (Wk|Wk).T @ kT         (bk dropped: softmax-invariant)
  V'  [2048, 65]  = vT.T @ Wv_aug + bias ; col 64 == 1.0 (denominator col)
  per kv-tile (128 kv rows x 512 q): scores.T -> PSUM, exp -> bf16 SBUF,
  V'.T @ P.T accumulated into [65, 512] per q-block.

Perf structure:
  - ScalarE is the floor engine (~8.5M exps/core at 1 elem/lane/cycle);
    it does exps only. scores PSUM = two 3-bank slots; one ACTIVATE per
    3-tile group (N=1536) ping-ponging the slots.
  - kv-tile pairs row-pack the PE (tile_position) so the 64-contraction
    scores MMs run concurrently; AV keeps full 128-contraction (row-
    splitting it doubles serialized LDWEIGHTS - measured regression).
  - PE warm-up matmuls on memset data flip the HAM clock gate to 2.4 GHz
    before the first input DMA lands.
  - DMA arrival order == need order per HWDGE ring FIFO; the early HBM
    window (~250 GB/s/core with small packets) is the head bottleneck.
    All five weight tensors ride ONE descriptor (bf16 "wall") at the
    head of the scalar ring, followed by q block 0 + V; sync carries the
    K quarters + the q tail + the output stores; the slow-starting SWDGE
    ring is kept empty. AVs lag scores by up to 6 exp-groups (tapering
    to 2) so late V quarters never stall the in-order PE queue.
"""

import numpy as np
import ml_dtypes

import concourse.bass as bass
import concourse.tile as tile
from concourse import mybir
from concourse.bass_utils import run_bass_kernel_spmd

BF16 = mybir.dt.bfloat16
F32 = mybir.dt.float32

B, S, D, E = 4, 4096, 512, 64
H = S                 # q rows per core (full sequence)
KS = S // 2           # kv rows per core (half sequence)
E1 = E + 1            # V' width (ones column appended)
NCH = D // 128        # contraction chunks (4)
NKV = KS // 128       # kv tiles per core (16)
QBLK = 512            # q columns per block
NBLK = H // QBLK      # 8
NT = NBLK * NKV       # global tile-unit count (128)
GRP = 3               # tile-units per exp group / psum slot
NGRP = (NT + GRP - 1) // GRP    # 43 (last group has 2 units)
N_CORES = 8

# weight-wall slice offsets (all bf16): wk, wq, bq, wv, bvb
_W_WK = 0
_W_WQ = 512
_W_BQ = 1024
_W_WV = 1536
_W_BVB = 1536 + NCH * E1
_W_END = _W_BVB + E1            # 1861


def _useg(U):
    """tile-unit U (0..127) -> (q-block, kv-tile)."""
    return divmod(U, NKV)


def _build_bass(split_waits: bool = True) -> bass.Bass:
    nc = bass.Bass()
    qT = nc.declare_dram_parameter("qT", [D, H], BF16, isOutput=False)
    kT = nc.declare_dram_parameter("kT", [D, KS], BF16, isOutput=False)
    vT = nc.declare_dram_parameter("vT", [D, KS], BF16, isOutput=False)
    wall = nc.declare_dram_parameter("wall", [128, _W_END], BF16, isOutput=False)
    out = nc.declare_dram_parameter("out", [E1, H], BF16, isOutput=True)

    with tile.TileContext(nc) as tc:
        _body(nc, tc, qT, kT, vT, wall, out)
    _prune_waits(nc)
    if split_waits:
        _split_multi_waits(nc)
    return nc


_NO_SPLIT_OPCODES = {"Drain", "EventSemaphore", "NoOp", "Call", "ISA",
                     "UnconditionalBranch"}


def _prune_waits(nc):
    """Drop provably-redundant sem waits before the wait-split pass.

    (a) A wait on a sem that is incremented ONLY by the waiting
        instruction's own engine is satisfied by in-order execution
        (tile never emits a forward-referencing wait - that would
        deadlock), so it can go.
    (b) A sem-ge-imm wait dominated by an earlier wait on the same
        engine for the same sem with a >= threshold (counters are
        monotone) can go.
    Fewer multi-wait instructions -> fewer injected EventSemaphore
    stalls on the ScalarE/PE queues."""
    incrementers = {}
    for blk in nc.m.functions[0].blocks:
        for inst in blk.instructions:
            si = inst.sync_info
            if si is not None:
                for u in (si.on_update or []):
                    incrementers.setdefault(u.id, set()).add(str(inst.engine))
    seen_max = {}
    dropped = 0
    for blk in nc.m.functions[0].blocks:
        for inst in blk.instructions:
            si = inst.sync_info
            if si is None or not si.on_wait:
                continue
            eng = str(inst.engine)
            keep = []
            for w in si.on_wait:
                nm = (w.ant_name or "")
                if (w.wait_mode != "sem-ge-imm" or w.wait_value is None
                        or nm.split("_")[0] not in
                        ("PE", "Activation", "DVE", "SP", "Pool")):
                    keep.append(w)
                    continue
                incs = incrementers.get(w.id)
                if incs is not None and incs == {eng}:
                    dropped += 1
                    continue
                prev = seen_max.get((eng, w.id))
                if prev is not None and prev >= w.wait_value:
                    dropped += 1
                    continue
                seen_max[(eng, w.id)] = max(prev or 0, w.wait_value)
                keep.append(w)
            if len(keep) != len(si.on_wait):
                inst.sync_info = mybir.SyncInfo(
                    on_wait=keep, on_update=list(si.on_update or []))
    return dropped


def _split_multi_waits(nc):
    """walrus (this toolchain) encodes at most ONE sem wait per TPB
    instruction (single NEURON_ISA_TPB_EVENTS slot) and refuses to compile
    instructions carrying more. Tile emits multi-wait sync_info freely, so
    split: keep the first wait on the instruction, hoist the rest onto
    standalone EventSemaphore waits just before it on the same engine."""
    n = 0
    for blk in nc.m.functions[0].blocks:
        new_insts = []
        for inst in blk.instructions:
            si = inst.sync_info
            if (si is not None and si.on_wait and len(si.on_wait) > 1
                    and inst.concise_opcode not in _NO_SPLIT_OPCODES):
                waits = list(si.on_wait)
                for w in waits[:-1]:
                    n += 1
                    es = mybir.InstEventSemaphore(
                        name=f"WSPLIT-{n}", ins=[], outs=[])
                    es.engine = inst.engine
                    es.sync_info = mybir.SyncInfo(on_wait=[w], on_update=[])
                    new_insts.append(es)
                inst.sync_info = mybir.SyncInfo(
                    on_wait=[waits[-1]], on_update=list(si.on_update))
            new_insts.append(inst)
        blk.instructions = new_insts
    return nc


def _body(nc, tc, qT, kT, vT, wall, out):
    with (
        tc.tile_pool(name="consts", bufs=1) as cst,
        tc.tile_pool(name="raw", bufs=1) as raw,
        tc.tile_pool(name="proj", bufs=1) as proj,
        tc.tile_pool(name="pt", bufs=12) as ptp,
        tc.tile_pool(name="ob", bufs=2) as obp,
        tc.tile_pool(name="sc", bufs=2, space="PSUM") as scp,
        tc.tile_pool(name="acc", bufs=1, space="PSUM") as accp,
        tc.tile_pool(name="pp", bufs=1, space="PSUM") as ppp,
    ):
        def load3d(eng, name, src, c0, c1):
            w = c1 - c0
            t = raw.tile([128, NCH, w], BF16, tag=name, name=name)
            eng.dma_start(
                out=t,
                in_=src[:, c0:c1].rearrange("(c p) w -> p c w", p=128))
            return t

        # gpsimd (SWDGE): warm-up memset; later only output stores
        warm = cst.tile([128, 256], BF16, tag="warm")
        nc.gpsimd.memset(warm, 0.0)

        # scalar ring (HWDGE): weight wall in ONE descriptor, q block 0,
        # the exp-table preload, q block 1, then all of V; afterwards the
        # ScalarE queue runs exps only.
        kqa = load3d(nc.scalar, "kqa", kT, 0, 256)
        wall_sb = cst.tile([128, _W_END], BF16, tag="wall")
        nc.scalar.dma_start(out=wall_sb, in_=wall[:, :])
        kq1 = load3d(nc.scalar, "kq1", kT, 512, 1024)
        scr = cst.tile([1, 8], F32, tag="scr")
        nc.scalar.activation(scr[:, :], warm[0:1, 0:8],
                             mybir.ActivationFunctionType.Exp)
        qt_b1 = load3d(nc.scalar, "qt_b1", qT, 512, 1024)
        vqs = [load3d(nc.scalar, f"vq{i}", vT, i * 512, (i + 1) * 512)
               for i in range(4)]

        # sync ring (HWDGE): K first half, q tail, K second half; the
        # per-block output stores ride this ring late (SWDGE is too slow)
        qt_b0 = load3d(nc.sync, "qt_b0", qT, 0, 512)
        kqb = load3d(nc.sync, "kqb", kT, 256, 512)
        kq2 = load3d(nc.sync, "kq2", kT, 1024, 1536)
        kq3 = load3d(nc.sync, "kq3", kT, 1536, 2048)
        qt_b2 = load3d(nc.sync, "qt_b2", qT, 1024, 1536)
        qt_r34 = load3d(nc.sync, "qt_r34", qT, 1536, 2560)
        qt_r567 = load3d(nc.sync, "qt_r567", qT, 2560, H)

        wk_sb = wall_sb[:, _W_WK:_W_WK + 512]
        wq_sb = wall_sb[:, _W_WQ:_W_WQ + 512]
        bq_sb = wall_sb[:, _W_BQ:_W_BQ + 512]
        wv_sb = wall_sb[:, _W_WV:_W_WV + NCH * E1]
        bvb_sb = wall_sb[:, _W_BVB:_W_BVB + E1]

        # PE warm-up: ~4us of matmuls on memset garbage flips the HAM
        # clock gate (1.2 -> 2.4 GHz) while the input DMAs are in flight.
        wacc = ppp.tile([128, QBLK], F32, tag="pp", name="wacc")
        for i in range(12):
            nc.tensor.matmul(wacc[:, 0:256], warm[:, 0:128], warm[:, :],
                             start=(i == 0), stop=(i == 11))

        def qt_slice(c, blk):
            if blk == 0:
                return qt_b0[:, c, :]
            if blk == 1:
                return qt_b1[:, c, :]
            if blk == 2:
                return qt_b2[:, c, :]
            if blk < 5:
                return qt_r34[:, c, (blk - 3) * 512:(blk - 2) * 512]
            return qt_r567[:, c, (blk - 5) * 512:(blk - 4) * 512]

        def k_moving(c, kt2b):      # 256-col raw-K slice for projection
            if kt2b == 0:
                return kqa[:, c, :]
            if kt2b == 1:
                return kqb[:, c, :]
            src = (kq1, kq1, kq2, kq2, kq3, kq3)[kt2b - 2]
            off = (kt2b % 2) * 256
            return src[:, c, off:off + 256]

        def vt_slice(c, t):         # 128-col V tile
            return vqs[t // 4][:, c, (t % 4) * 128:(t % 4 + 1) * 128]

        # projected tensors; Q.T/K.T rows 64..127 duplicate rows 0..63 (via
        # column-duplicated weights) so scores matmuls can row-pack.
        QT2 = proj.tile([128, H], BF16, tag="QT2")
        KT2 = proj.tile([128, KS], BF16, tag="KT2")
        Vp = proj.tile([128, NKV, E1], BF16, tag="Vp")

        # AV accumulators ping-pong between the acc bank (blocks
        # 0,1,3,5,7) and the pp bank (blocks 2,4,6) so a block's first AV
        # never waits on the previous block's epilogue drain; q_proj picks
        # whichever bank is free in its emission window.
        def q_pool(blk):
            return accp if blk in (0, 1, 3, 5, 7) else ppp

        def q_proj(blk, pool=None):
            pool = ppp if pool is None else pool
            tg = "acc" if pool is accp else "pp"
            acc = pool.tile([128, QBLK], F32, tag=tg, name="qp")
            sl = slice(blk * QBLK, (blk + 1) * QBLK)
            for c in range(NCH):
                nc.tensor.matmul(
                    acc[:, :], wq_sb[:, c * 128:(c + 1) * 128],
                    qt_slice(c, blk),
                    start=(c == 0), stop=(c == NCH - 1))
            nc.vector.tensor_add(QT2[:, sl], acc[:, :], bq_sb[:, :])

        def k_proj(kt2b):           # 256-col K projection sub-block
            acc = ppp.tile([128, QBLK], F32, tag="pp", name="kp")
            sl = slice(kt2b * 256, (kt2b + 1) * 256)
            for c in range(NCH):
                nc.tensor.matmul(
                    acc[:, 0:256], wk_sb[:, c * 128:(c + 1) * 128],
                    k_moving(c, kt2b),
                    start=(c == 0), stop=(c == NCH - 1))
            nc.vector.tensor_copy(KT2[:, sl], acc[:, 0:256])

        def v_proj(t):
            acc = ppp.tile([128, QBLK], F32, tag="pp", name="vp")
            for c in range(NCH):
                nc.tensor.matmul(
                    acc[:, 0:E1], vt_slice(c, t),
                    wv_sb[:, c * E1:(c + 1) * E1],
                    start=(c == 0), stop=(c == NCH - 1))
            nc.vector.tensor_add(Vp[:, t, :], acc[:, 0:E1], bvb_sb[:, :])

        # --- attention over tile-units U (see _useg). Groups of GRP=3
        # units share one psum slot + one exp ACTIVATE.
        state = {"sc": None, "acc": None}
        pt_of = {}

        def scores(U):
            blk, t = _useg(U)
            p = U % GRP
            if p == 0:
                state["sc"] = scp.tile([128, GRP * QBLK], F32, tag="sc",
                                       name="sc")
            sq = slice(blk * QBLK, (blk + 1) * QBLK)
            half = U % 2
            nc.tensor.matmul(
                state["sc"][:, p * QBLK:(p + 1) * QBLK],
                KT2[half * E:(half + 1) * E, t * 128:(t + 1) * 128],
                QT2[half * E:(half + 1) * E, sq],
                start=True, stop=True, tile_position=(half * E, 0))

        def exp_group(g):
            lastU = min((g + 1) * GRP, NT) - 1
            n = (lastU % GRP) + 1
            pt = ptp.tile([128, GRP * QBLK], BF16, tag="pt", name="pt")
            nc.scalar.activation(
                pt[:, 0:n * QBLK], state["sc"][:, 0:n * QBLK],
                mybir.ActivationFunctionType.Exp, scale=0.125)
            pt_of[g] = pt

        def epilogue(blk):
            sq = slice(blk * QBLK, (blk + 1) * QBLK)
            ob = obp.tile([E1, QBLK], BF16, tag="ob", name="ob")
            nc.vector.tensor_copy(ob[:, :], state["acc"][:, :])
            nc.sync.dma_start(out=out[:, sq], in_=ob[:, :])

        def av(U):
            blk, t = _useg(U)
            if t == 0:
                if blk > 0:
                    epilogue(blk - 1)
                pool = accp if blk in (0, 1, 3, 5, 7) else ppp
                tg = "acc" if pool is accp else "pp"
                state["acc"] = pool.tile([E1, QBLK], F32, tag=tg,
                                         name="acc")
            nc.tensor.matmul(
                state["acc"][:, :], Vp[:, t, :],
                pt_of[U // GRP][:, (U % GRP) * QBLK:(U % GRP + 1) * QBLK],
                start=(t == 0), stop=(t == NKV - 1))

        def sc_group(g):
            for U in range(g * GRP, min((g + 1) * GRP, NT)):
                scores(U)
            exp_group(g)

        def av_group(g):
            for U in range(g * GRP, min((g + 1) * GRP, NT)):
                av(U)

        # --- schedule. K/Q-critical chain first; AVs lag scores by up
        # to 6 groups while V arrives, tapering to 2 near the end. The
        # AV accumulator alternates acc/pp banks per block; q_proj slots
        # into whichever bank is free in its emission window.
        k_proj(0)
        q_proj(0, pool=accp)
        scores(0); scores(1)
        k_proj(1)
        scores(2); exp_group(0)
        k_proj(2); k_proj(3)
        sc_group(1)
        k_proj(4); k_proj(5)
        sc_group(2)
        k_proj(6); k_proj(7)
        sc_group(3)
        sc_group(4)
        q_proj(1, pool=accp)
        sc_group(5)
        sc_group(6)
        v_proj(0); v_proj(1); v_proj(2)

        VP_AT = {7: (3, 4, 5), 8: (6, 7, 8), 9: (9, 10, 11),
                 10: (12, 13, 14, 15)}
        QP_END = {9: 2, 14: 3, 20: 4, 25: 5, 30: 6, 35: 7}
        next_av = 0
        for g in range(7, NGRP):
            for t in VP_AT.get(g, ()):
                v_proj(t)
            sc_group(g)
            lag = 6 if g < 12 else 5 if g < 13 else 4 if g < 26 else \
                3 if g < 29 else 2
            while next_av <= g - lag:
                av_group(next_av)
                next_av += 1
            if g in QP_END:
                q_proj(QP_END[g], pool=q_pool(QP_END[g]))
        while next_av < NGRP:
            av_group(next_av)
            next_av += 1
        epilogue(NBLK - 1)


_CACHED_NC = None


def _get_nc():
    global _CACHED_NC
    if _CACHED_NC is None:
        _CACHED_NC = _build_bass()
    return _CACHED_NC


def _swizzle_w(w: np.ndarray) -> np.ndarray:
    """[512, width] -> [128, NCH*width] with chunk-major free dim."""
    width = w.shape[1]
    return np.ascontiguousarray(
        w.reshape(NCH, 128, width).transpose(1, 0, 2).reshape(128, NCH * width)
    ).astype(ml_dtypes.bfloat16)


def _make_in_maps(q, k, v, Wq, bq, Wk, bk, Wv, bv):
    del bk  # constant along the kv axis -> softmax-invariant, dropped
    bf = ml_dtypes.bfloat16
    wq_d = np.concatenate([np.asarray(Wq, np.float32)] * 2, axis=1)
    wk_d = np.concatenate([np.asarray(Wk, np.float32)] * 2, axis=1)
    wq_s = _swizzle_w(wq_d)
    wk_s = _swizzle_w(wk_d)
    wv_aug = np.concatenate(
        [np.asarray(Wv, np.float32), np.zeros((D, 1), np.float32)], axis=1
    )
    wv_s = _swizzle_w(wv_aug)
    bq_col = np.asarray(bq, np.float32).reshape(E, 1)
    bq_a = np.ascontiguousarray(np.broadcast_to(
        np.concatenate([bq_col, bq_col], axis=0), (2 * E, QBLK))).astype(bf)
    bvb_row = np.concatenate([np.asarray(bv, np.float32), [1.0]])
    bvb_a = np.ascontiguousarray(
        np.broadcast_to(bvb_row, (128, E1))).astype(bf)
    wall = np.concatenate([wk_s, wq_s, bq_a, wv_s, bvb_a], axis=1)
    assert wall.shape == (128, _W_END)

    in_maps = []
    for core in range(N_CORES):
        b, h = core // 2, core % 2
        kh = np.asarray(k[b, h * KS:(h + 1) * KS, :], np.float32)
        vh = np.asarray(v[b, h * KS:(h + 1) * KS, :], np.float32)
        in_maps.append({
            "qT": np.ascontiguousarray(np.asarray(q[b], np.float32).T).astype(bf),
            "kT": np.ascontiguousarray(kh.T).astype(bf),
            "vT": np.ascontiguousarray(vh.T).astype(bf),
            "wall": wall,
        })
    return in_maps


def _unshard(results) -> np.ndarray:
    final = np.empty((B, S, E), np.float32)
    for b in range(B):
        o = (np.asarray(results[2 * b]["out"], np.float32)
             + np.asarray(results[2 * b + 1]["out"], np.float32))  # [65, S]
        final[b] = (o[:E] / o[E:E + 1]).T
    return final


def kernel(q, k, v, Wq, bq, Wk, bk, Wv, bv, _trace=False):
    nc = _get_nc()
    in_maps = _make_in_maps(q, k, v, Wq, bq, Wk, bk, Wv, bv)
    res = run_bass_kernel_spmd(nc, in_maps, core_ids=list(range(N_CORES)),
                               trace=_trace)
    outp = _unshard(res.results)
    if _trace:
        kernel.last_result = res
    return outp
